# revision 31
# baseline (speedup 1.0000x reference)
"""Multi-head attention (B=4, N=2048, D=512, H=8, Dh=64) on 8 trn2 cores.

Sharding: core c handles batch b = c//2 and head-group hg = c%2 (4 heads =
2 pairs).  Each core computes its batch's attention output for its 4 heads
plus the partial output projection; the host sums the two head-group
partials per batch.

v3: every matmul in the attention steady state runs in 64-row PE-tiling
mode (tiles T0/T8) so the array never pays a mode-switch drain (~105ns),
and row-tile pairs stream concurrently (2x) where the contraction is 64:
 - Scores S^T per pair: T0 computes the even head, T8 the odd head,
   concurrently, into the two banks of one [128, 1024] PSUM tile.
 - exp on the scalar engine per jt ([128, 1024], ~1020ns each).
 - PV splits its 128-contraction into T0/T8 j-halves (concurrent), with a
   [V|ones] 65-column stationary; the two half-results and half-denominators
   are combined by single fused DVE adds on PSUM eviction.
 - Denominators are batch-reciprocaled ([16,128] DVE op per i-block) and
   broadcast back via a DRAM bounce; normalization and the output
   projection for block ib run interleaved under block ib+1's attention.
All PE operands bf16 (f32 PSUM accumulation); QKV projection runs in full
128-row mode as a separate phase.
"""

import sys

for p in ("/opt/trn_rl_repo", "/root/.axon_site/_ro/trn_rl_repo"):
    if p not in sys.path:
        sys.path.insert(0, p)

from contextlib import ExitStack

import numpy as np
import ml_dtypes

import concourse.bass as bass
import concourse.mybir as mybir
import concourse.tile as tile
from concourse import bacc
from concourse.bass_utils import run_bass_kernel_spmd

F32 = mybir.dt.float32
BF16 = mybir.dt.bfloat16
AF = mybir.ActivationFunctionType
BF16NP = ml_dtypes.bfloat16

N_CORES = 8
B, N, D = 4, 2048, 512
HEADS = 8
DH = 64
SCALE = DH**-0.5
HPC = 4  # heads per core (2 pairs)
P = 128
NDT = D // P  # 4 d-tiles
NJT = N // P  # 16 j-tiles
IB = 512  # i-block
NIB = N // IB  # 4 i-blocks

N_REPS = 1  # replications of the whole body inside one NEFF (for timing)


def build_program(n_reps: int = N_REPS):
    nc = bacc.Bacc("TRN2", target_bir_lowering=False, debug=False,
                   num_devices=N_CORES)
    xT = nc.dram_tensor("xT", [D, N], BF16, kind="ExternalInput").ap()
    wqk = nc.dram_tensor("wqk", [D, 2 * HPC * DH], BF16, kind="ExternalInput").ap()
    wv = nc.dram_tensor("wv", [D, HPC * DH], BF16, kind="ExternalInput").ap()
    wo = nc.dram_tensor("wo", [HPC * DH, D], BF16, kind="ExternalInput").ap()
    bias = nc.dram_tensor("bias", [D, 1], F32, kind="ExternalInput").ap()
    yT = nc.dram_tensor("yT", [D, N], F32, kind="ExternalOutput").ap()
    # DRAM bounce for denominator reshape + partition broadcast
    den_d = nc.dram_tensor("den_d", [NIB, HPC * IB], BF16).ap()
    rden_d = nc.dram_tensor("rden_d", [NIB, HPC * IB], BF16).ap()


    with tile.TileContext(nc) as tc, ExitStack() as ctx:
        sb = ctx.enter_context(tc.tile_pool(name="sb", bufs=1))
        if n_reps > 1:
            ctx.enter_context(tc.For_i(0, n_reps, 1))

        for _rep in range(1):
            # preload the exp activation table while the input DMAs run
            warm = sb.tile([1, 16], F32, tag="warm", bufs=1)
            nc.vector.memset(warm, 0.0)
            nc.scalar.activation(warm, warm, AF.Exp, scale=1.0)

            # ---------------- phase 1: load + QKV projection ----------------
            wqk_sb = []
            wv_sb = []
            bias_sb = []
            for dt in range(NDT):
                t = sb.tile([P, 2 * HPC * DH], BF16, tag="wqk", bufs=NDT)
                nc.sync.dma_start(out=t, in_=wqk[dt * P:(dt + 1) * P, :])
                wqk_sb.append(t)
                t = sb.tile([P, HPC * DH], BF16, tag="wv", bufs=NDT)
                nc.sync.dma_start(out=t, in_=wv[dt * P:(dt + 1) * P, :])
                wv_sb.append(t)
                t = sb.tile([P, 1], F32, tag="bias", bufs=NDT)
                nc.sync.dma_start(out=t, in_=bias[dt * P:(dt + 1) * P, :])
                bias_sb.append(t)
            wo_sb = []
            for h in range(HPC):
                t = sb.tile([DH, D], BF16, tag="wo", bufs=HPC)
                nc.sync.dma_start(out=t, in_=wo[h * DH:(h + 1) * DH, :])
                wo_sb.append(t)

            xt_sb = []
            for dt in range(NDT):
                t = sb.tile([P, N], BF16, tag="xt", bufs=NDT)
                nc.sync.dma_start(out=t, in_=xT[dt * P:(dt + 1) * P, :])
                xt_sb.append(t)

            # Q^T/K^T tiles [128, N]; rows 0:64 even head of pair, 64:128 odd.
            # et: 0 = Q pair0, 1 = Q pair1, 2 = K pair0, 3 = K pair1
            qkt_sb = []
            with tc.tile_pool(name="ps1", bufs=1, space="PSUM") as ps1:
                for et in range(4):
                    t = sb.tile([P, N], BF16, tag="qkt", bufs=4)
                    qkt_sb.append(t)
                    for nb in range(NIB):
                        pq = ps1.tile([P, IB], F32, tag="qk", bufs=4)
                        for dt in range(NDT):
                            nc.tensor.matmul(
                                pq,
                                lhsT=wqk_sb[dt][:, et * P:(et + 1) * P],
                                rhs=xt_sb[dt][:, nb * IB:(nb + 1) * IB],
                                start=(dt == 0), stop=(dt == NDT - 1),
                            )
                        nc.vector.tensor_copy(t[:, nb * IB:(nb + 1) * IB], pq)

                # V natural [n, e] with a ones column per head: [128, 4*65]
                v_sb = []
                for nt in range(NJT):
                    t = sb.tile([P, HPC * (DH + 1)], BF16, tag="v", bufs=NJT)
                    v_sb.append(t)
                    pv = ps1.tile([P, HPC * DH], F32, tag="v", bufs=2)
                    for dt in range(NDT):
                        nc.tensor.matmul(
                            pv,
                            lhsT=xt_sb[dt][:, nt * P:(nt + 1) * P],
                            rhs=wv_sb[dt],
                            start=(dt == 0), stop=(dt == NDT - 1),
                        )
                    nc.vector.tensor_copy(
                        t.rearrange("p (h c) -> p h c", c=DH + 1)[:, :, 0:DH],
                        pv.rearrange("p (h c) -> p h c", c=DH),
                    )
                    nc.vector.memset(
                        t.rearrange("p (h c) -> p h c", c=DH + 1)[:, :, DH:DH + 1],
                        1.0,
                    )

            # ---------------- phase 2: attention (all 64-row mode) ----------
            # O^T per head [65, N]: rows 0:64 = O, row 64 = softmax
            # denominator (rides along in the PSUM eviction). Separate tiles
            # per head keep the dependency ranges narrow.
            ot_sb = [sb.tile([DH + 1, N], BF16, tag="ot", bufs=HPC,
                             name=f"ot{_h}") for _h in range(HPC)]

            def ot_o(h, ib):
                return ot_sb[h][0:DH, ib * IB:(ib + 1) * IB]

            units = [(ib, pair, jt) for ib in range(NIB)
                     for pair in range(2) for jt in range(NJT)]
            su_t = {}

            with tc.tile_pool(name="ps2", bufs=1, space="PSUM") as ps2:
                def emit_scores(k):
                    ib, pair, jt = units[k]
                    su = ps2.tile([P, 2 * IB], F32, tag="s", bufs=2,
                                  name=f"su{k}")
                    for par in range(2):
                        lo, hi = par * DH, (par + 1) * DH
                        nc.tensor.matmul(
                            su[:, par * IB:(par + 1) * IB],
                            lhsT=qkt_sb[2 + pair][lo:hi, jt * P:(jt + 1) * P],
                            rhs=qkt_sb[pair][lo:hi, ib * IB:(ib + 1) * IB],
                            start=True, stop=True,
                        )
                    su_t[k] = su

                def emit_proj_half(ibp, half):
                    # output projection for 2 of the 4 d-tiles of block ibp,
                    # borrowing a su-tag PSUM slot ([128, 1024] = 2 banks).
                    yp2 = ps2.tile([P, 2 * IB], F32, tag="s", bufs=2,
                                   name=f"yp{ibp}_{half}")
                    yt_t = sb.tile([P, 2 * IB], F32, tag="yt", bufs=2)
                    for i in range(2):
                        dt4 = 2 * half + i
                        for h in range(HPC):
                            nc.tensor.matmul(
                                yp2[:, i * IB:(i + 1) * IB],
                                lhsT=wo_sb[h][:, dt4 * P:(dt4 + 1) * P],
                                rhs=ot_o(h, ibp),
                                start=(h == 0), stop=(h == HPC - 1),
                            )
                        nc.vector.tensor_scalar_add(
                            yt_t[:, i * IB:(i + 1) * IB],
                            yp2[:, i * IB:(i + 1) * IB], bias_sb[dt4])
                        nc.sync.dma_start(
                            out=yT[dt4 * P:(dt4 + 1) * P,
                                   ibp * IB:(ibp + 1) * IB],
                            in_=yt_t[:, i * IB:(i + 1) * IB],
                        )

                def emit_chain(ib):
                    # denominator reciprocal + normalization for block ib:
                    # per-head den rows -> DRAM, reciprocal on [16,128], one
                    # broadcast DMA for all heads, gpsimd muls.
                    for h in range(HPC):
                        nc.sync.dma_start(
                            out=den_d[ib:ib + 1, h * IB:(h + 1) * IB],
                            in_=ot_sb[h][DH:DH + 1, ib * IB:(ib + 1) * IB])
                    rc2 = sb.tile([4 * HPC, P], BF16, tag="rc2", bufs=2)
                    nc.sync.dma_start(
                        out=rc2,
                        in_=den_d[ib:ib + 1, :].rearrange(
                            "a (q p) -> (a q) p", p=P))
                    with nc.allow_low_precision(
                            reason="bf16 reciprocal of softmax denom, 0.4% ok"):
                        nc.vector.reciprocal(rc2, rc2)
                    nc.sync.dma_start(
                        out=rden_d[ib:ib + 1, :].rearrange(
                            "a (q p) -> (a q) p", p=P),
                        in_=rc2)
                    rb = sb.tile([DH, HPC * IB], BF16, tag="rb", bufs=2)
                    src = rden_d[ib:ib + 1, :]
                    bcast = bass.AP(
                        tensor=src.tensor, offset=src.offset,
                        ap=[[0, DH]] + [list(d) for d in src.ap[-1:]],
                    )
                    nc.sync.dma_start(out=rb, in_=bcast)
                    for h in range(HPC):
                        nc.gpsimd.tensor_mul(ot_o(h, ib), ot_o(h, ib),
                                             rb[:, h * IB:(h + 1) * IB])

                emit_scores(0)
                pv_t = None
                for k, (ib, pair, jt) in enumerate(units):
                    if jt == 0:
                        pv_t = [ps2.tile([P, IB], F32, tag="pv", bufs=4,
                                         name=f"pv{ib}_{pair}_{_k}")
                                for _k in range(4)]
                    if k + 1 < len(units):
                        emit_scores(k + 1)
                    es = sb.tile([P, 2 * IB], BF16, tag="big", bufs=8)
                    nc.scalar.activation(es, su_t.pop(k), AF.Exp, scale=SCALE)
                    for par in range(2):
                        h = 2 * pair + par
                        for jh in range(2):
                            nc.tensor.matmul(
                                pv_t[2 * par + jh][0:DH + 1, :],
                                lhsT=v_sb[jt][jh * DH:(jh + 1) * DH,
                                              h * (DH + 1):(h + 1) * (DH + 1)],
                                rhs=es[jh * DH:(jh + 1) * DH,
                                       par * IB:(par + 1) * IB],
                                start=(jt == 0), stop=(jt == NJT - 1),
                            )
                    if pair == 0 and ib >= 1 and jt in (6, 11):
                        emit_proj_half(ib - 1, 0 if jt == 6 else 1)
                    if jt == NJT - 1:
                        # evict this block's PV accumulators (O and den rows)
                        for par in range(2):
                            h = 2 * pair + par
                            a, b = pv_t[2 * par], pv_t[2 * par + 1]
                            osl = ot_sb[h][:, ib * IB:(ib + 1) * IB]
                            nc.vector.tensor_copy(osl, a[0:DH + 1, :])
                            with nc.allow_low_precision(
                                    reason="bf16 j-half merge, 0.4% ok"):
                                nc.vector.tensor_add(osl, osl, b[0:DH + 1, :])
                        if pair == 1:
                            emit_chain(ib)
                emit_proj_half(NIB - 1, 0)
                emit_proj_half(NIB - 1, 1)

    nc.finalize()
    return nc


_nc_cache = {}


def _get_program(n_reps):
    if n_reps not in _nc_cache:
        _nc_cache[n_reps] = build_program(n_reps)
    return _nc_cache[n_reps]


def make_in_maps(x, w_qkv, w_out, b_out):
    x = np.asarray(x, np.float32)
    w_qkv = np.asarray(w_qkv, np.float32)
    w_out = np.asarray(w_out, np.float32)
    b_out = np.asarray(b_out, np.float32)
    in_maps = []
    for core in range(N_CORES):
        b, hg = core // 2, core % 2
        s = 256 * hg
        wq = w_qkv[s:s + 256]
        wk = w_qkv[512 + s:512 + s + 256]
        wv_ = w_qkv[1024 + s:1024 + s + 256]
        in_maps.append({
            "xT": np.ascontiguousarray(x[b].T).astype(BF16NP),
            "wqk": np.ascontiguousarray(np.concatenate([wq, wk], 0).T).astype(BF16NP),
            "wv": np.ascontiguousarray(wv_.T).astype(BF16NP),
            "wo": np.ascontiguousarray(w_out[:, s:s + 256].T).astype(BF16NP),
            "bias": np.ascontiguousarray((b_out / 2).reshape(D, 1)),
        })
    return in_maps


def kernel(x, w_qkv, w_out, b_out):
    nc = _get_program(N_REPS)
    in_maps = make_in_maps(x, w_qkv, w_out, b_out)
    res = run_bass_kernel_spmd(nc, in_maps, list(range(N_CORES)))
    out = np.empty((B, N, D), np.float32)
    for b in range(B):
        out[b] = (res.results[2 * b]["yT"] + res.results[2 * b + 1]["yT"]).T
    return out


if __name__ == "__main__":
    nc = build_program(1)
    print("built OK; instructions:",
          sum(len(blk.instructions) for f in nc.m.functions for blk in f.blocks))


# revision 32
# speedup vs baseline: 1.0118x; 1.0118x over previous
"""Multi-head attention (B=4, N=2048, D=512, H=8, Dh=64) on 8 trn2 cores.

Sharding: core c handles batch b = c//2 and head-group hg = c%2 (4 heads =
2 pairs).  Each core computes its batch's attention output for its 4 heads
plus the partial output projection; the host sums the two head-group
partials per batch.

v3: every matmul in the attention steady state runs in 64-row PE-tiling
mode (tiles T0/T8) so the array never pays a mode-switch drain (~105ns),
and row-tile pairs stream concurrently (2x) where the contraction is 64:
 - Scores S^T per pair: T0 computes the even head, T8 the odd head,
   concurrently, into the two banks of one [128, 1024] PSUM tile.
 - exp on the scalar engine per jt ([128, 1024], ~1020ns each).
 - PV splits its 128-contraction into T0/T8 j-halves (concurrent), with a
   [V|ones] 65-column stationary; the two half-results and half-denominators
   are combined by single fused DVE adds on PSUM eviction.
 - Denominators are batch-reciprocaled ([16,128] DVE op per i-block) and
   broadcast back via a DRAM bounce; normalization and the output
   projection for block ib run interleaved under block ib+1's attention.
All PE operands bf16 (f32 PSUM accumulation); QKV projection runs in full
128-row mode as a separate phase.
"""

import sys

for p in ("/opt/trn_rl_repo", "/root/.axon_site/_ro/trn_rl_repo"):
    if p not in sys.path:
        sys.path.insert(0, p)

from contextlib import ExitStack

import numpy as np
import ml_dtypes

import concourse.bass as bass
import concourse.mybir as mybir
import concourse.tile as tile
from concourse import bacc
from concourse.bass_utils import run_bass_kernel_spmd

F32 = mybir.dt.float32
BF16 = mybir.dt.bfloat16
AF = mybir.ActivationFunctionType
BF16NP = ml_dtypes.bfloat16

N_CORES = 8
B, N, D = 4, 2048, 512
HEADS = 8
DH = 64
SCALE = DH**-0.5
HPC = 4  # heads per core (2 pairs)
P = 128
NDT = D // P  # 4 d-tiles
NJT = N // P  # 16 j-tiles
IB = 512  # i-block
NIB = N // IB  # 4 i-blocks

N_REPS = 1  # replications of the whole body inside one NEFF (for timing)


def build_program(n_reps: int = N_REPS):
    nc = bacc.Bacc("TRN2", target_bir_lowering=False, debug=False,
                   num_devices=N_CORES)
    xT = nc.dram_tensor("xT", [D, N], BF16, kind="ExternalInput").ap()
    wqk = nc.dram_tensor("wqk", [D, 2 * HPC * DH], BF16, kind="ExternalInput").ap()
    wv = nc.dram_tensor("wv", [D, HPC * DH], BF16, kind="ExternalInput").ap()
    wo = nc.dram_tensor("wo", [HPC * DH, D], BF16, kind="ExternalInput").ap()
    bias = nc.dram_tensor("bias", [D, 1], F32, kind="ExternalInput").ap()
    yT = nc.dram_tensor("yT", [D, N], F32, kind="ExternalOutput").ap()
    # DRAM bounce for denominator reshape + partition broadcast
    den_d = nc.dram_tensor("den_d", [NIB, HPC * IB], BF16).ap()
    rden_d = nc.dram_tensor("rden_d", [NIB, HPC * IB], BF16).ap()


    with tile.TileContext(nc) as tc, ExitStack() as ctx:
        sb = ctx.enter_context(tc.tile_pool(name="sb", bufs=1))
        if n_reps > 1:
            ctx.enter_context(tc.For_i(0, n_reps, 1))

        for _rep in range(1):
            # preload the exp activation table while the input DMAs run
            warm = sb.tile([1, 16], F32, tag="warm", bufs=1)
            nc.vector.memset(warm, 0.0)
            nc.scalar.activation(warm, warm, AF.Exp, scale=1.0)

            # ---------------- phase 1: load + QKV projection ----------------
            wqk_sb = []
            wv_sb = []
            bias_sb = []
            for dt in range(NDT):
                t = sb.tile([P, 2 * HPC * DH], BF16, tag="wqk", bufs=NDT)
                nc.sync.dma_start(out=t, in_=wqk[dt * P:(dt + 1) * P, :])
                wqk_sb.append(t)
                t = sb.tile([P, HPC * DH], BF16, tag="wv", bufs=NDT)
                nc.sync.dma_start(out=t, in_=wv[dt * P:(dt + 1) * P, :])
                wv_sb.append(t)
                t = sb.tile([P, 1], F32, tag="bias", bufs=NDT)
                nc.sync.dma_start(out=t, in_=bias[dt * P:(dt + 1) * P, :])
                bias_sb.append(t)
            wo_sb = []
            for h in range(HPC):
                t = sb.tile([DH, D], BF16, tag="wo", bufs=HPC)
                nc.sync.dma_start(out=t, in_=wo[h * DH:(h + 1) * DH, :])
                wo_sb.append(t)

            xt_sb = []
            for dt in range(NDT):
                t = sb.tile([P, N], BF16, tag="xt", bufs=NDT)
                nc.sync.dma_start(out=t, in_=xT[dt * P:(dt + 1) * P, :])
                xt_sb.append(t)

            # Q^T/K^T tiles [128, N]; rows 0:64 even head of pair, 64:128 odd.
            # et: 0 = Q pair0, 1 = Q pair1, 2 = K pair0, 3 = K pair1
            qkt_sb = []
            with tc.tile_pool(name="ps1", bufs=1, space="PSUM") as ps1:
                for et in range(4):
                    t = sb.tile([P, N], BF16, tag="qkt", bufs=4)
                    qkt_sb.append(t)
                    for nb in range(NIB):
                        pq = ps1.tile([P, IB], F32, tag="qk", bufs=4)
                        for dt in range(NDT):
                            nc.tensor.matmul(
                                pq,
                                lhsT=wqk_sb[dt][:, et * P:(et + 1) * P],
                                rhs=xt_sb[dt][:, nb * IB:(nb + 1) * IB],
                                start=(dt == 0), stop=(dt == NDT - 1),
                            )
                        nc.vector.tensor_copy(t[:, nb * IB:(nb + 1) * IB], pq)

                # V natural [n, e] with a ones column per head: [128, 4*65]
                v_sb = []
                for nt in range(NJT):
                    t = sb.tile([P, HPC * (DH + 1)], BF16, tag="v", bufs=NJT)
                    v_sb.append(t)
                    pv = ps1.tile([P, HPC * DH], F32, tag="v", bufs=2)
                    for dt in range(NDT):
                        nc.tensor.matmul(
                            pv,
                            lhsT=xt_sb[dt][:, nt * P:(nt + 1) * P],
                            rhs=wv_sb[dt],
                            start=(dt == 0), stop=(dt == NDT - 1),
                        )
                    nc.vector.tensor_copy(
                        t.rearrange("p (h c) -> p h c", c=DH + 1)[:, :, 0:DH],
                        pv.rearrange("p (h c) -> p h c", c=DH),
                    )
                    nc.vector.memset(
                        t.rearrange("p (h c) -> p h c", c=DH + 1)[:, :, DH:DH + 1],
                        1.0,
                    )

            # ---------------- phase 2: attention (all 64-row mode) ----------
            # O^T per head [65, N]: rows 0:64 = O, row 64 = softmax
            # denominator (rides along in the PSUM eviction). Separate tiles
            # per head keep the dependency ranges narrow.
            ot_sb = [sb.tile([DH + 1, N], BF16, tag="ot", bufs=HPC,
                             name=f"ot{_h}") for _h in range(HPC)]

            def ot_o(h, ib):
                return ot_sb[h][0:DH, ib * IB:(ib + 1) * IB]

            units = [(ib, pair, jt) for ib in range(NIB)
                     for pair in range(2) for jt in range(NJT)]
            su_t = {}

            with tc.tile_pool(name="ps2", bufs=1, space="PSUM") as ps2:
                def emit_scores(k):
                    ib, pair, jt = units[k]
                    su = ps2.tile([P, 2 * IB], F32, tag="s", bufs=2,
                                  name=f"su{k}")
                    for par in range(2):
                        lo, hi = par * DH, (par + 1) * DH
                        nc.tensor.matmul(
                            su[:, par * IB:(par + 1) * IB],
                            lhsT=qkt_sb[2 + pair][lo:hi, jt * P:(jt + 1) * P],
                            rhs=qkt_sb[pair][lo:hi, ib * IB:(ib + 1) * IB],
                            start=True, stop=True,
                        )
                    su_t[k] = su

                def emit_proj_half(ibp, half):
                    # output projection for 2 of the 4 d-tiles of block ibp,
                    # borrowing a su-tag PSUM slot ([128, 1024] = 2 banks).
                    yp2 = ps2.tile([P, 2 * IB], F32, tag="s", bufs=2,
                                   name=f"yp{ibp}_{half}")
                    yt_t = sb.tile([P, 2 * IB], F32, tag="yt", bufs=2)
                    for i in range(2):
                        dt4 = 2 * half + i
                        for h in range(HPC):
                            nc.tensor.matmul(
                                yp2[:, i * IB:(i + 1) * IB],
                                lhsT=wo_sb[h][:, dt4 * P:(dt4 + 1) * P],
                                rhs=ot_o(h, ibp),
                                start=(h == 0), stop=(h == HPC - 1),
                            )
                        nc.vector.tensor_scalar_add(
                            yt_t[:, i * IB:(i + 1) * IB],
                            yp2[:, i * IB:(i + 1) * IB], bias_sb[dt4])
                        nc.sync.dma_start(
                            out=yT[dt4 * P:(dt4 + 1) * P,
                                   ibp * IB:(ibp + 1) * IB],
                            in_=yt_t[:, i * IB:(i + 1) * IB],
                        )

                def emit_chain(ib):
                    # denominator reciprocal + normalization for block ib:
                    # per-head den rows -> DRAM, reciprocal on [16,128], one
                    # broadcast DMA for all heads, gpsimd muls.
                    for h in range(HPC):
                        nc.sync.dma_start(
                            out=den_d[ib:ib + 1, h * IB:(h + 1) * IB],
                            in_=ot_sb[h][DH:DH + 1, ib * IB:(ib + 1) * IB])
                    rc2 = sb.tile([4 * HPC, P], BF16, tag="rc2", bufs=2)
                    nc.sync.dma_start(
                        out=rc2,
                        in_=den_d[ib:ib + 1, :].rearrange(
                            "a (q p) -> (a q) p", p=P))
                    with nc.allow_low_precision(
                            reason="bf16 reciprocal of softmax denom, 0.4% ok"):
                        nc.vector.reciprocal(rc2, rc2)
                    nc.sync.dma_start(
                        out=rden_d[ib:ib + 1, :].rearrange(
                            "a (q p) -> (a q) p", p=P),
                        in_=rc2)
                    for h in range(HPC):
                        rb = sb.tile([DH, IB], BF16, tag="rb", bufs=4)
                        src = rden_d[ib, h * IB:(h + 1) * IB]
                        bcast = bass.AP(
                            tensor=src.tensor, offset=src.offset,
                            ap=[[0, DH]] + [list(d) for d in src.ap[-1:]],
                        )
                        nc.sync.dma_start(out=rb, in_=bcast)
                        nc.gpsimd.tensor_mul(ot_o(h, ib), ot_o(h, ib), rb)

                emit_scores(0)
                pv_t = None
                for k, (ib, pair, jt) in enumerate(units):
                    if jt == 0:
                        pv_t = [ps2.tile([P, IB], F32, tag="pv", bufs=4,
                                         name=f"pv{ib}_{pair}_{_k}")
                                for _k in range(4)]
                    if k + 1 < len(units):
                        emit_scores(k + 1)
                    es = sb.tile([P, 2 * IB], BF16, tag="big", bufs=8)
                    nc.scalar.activation(es, su_t.pop(k), AF.Exp, scale=SCALE)
                    for par in range(2):
                        h = 2 * pair + par
                        for jh in range(2):
                            nc.tensor.matmul(
                                pv_t[2 * par + jh][0:DH + 1, :],
                                lhsT=v_sb[jt][jh * DH:(jh + 1) * DH,
                                              h * (DH + 1):(h + 1) * (DH + 1)],
                                rhs=es[jh * DH:(jh + 1) * DH,
                                       par * IB:(par + 1) * IB],
                                start=(jt == 0), stop=(jt == NJT - 1),
                            )
                    if pair == 0 and ib >= 1 and jt in (6, 11):
                        emit_proj_half(ib - 1, 0 if jt == 6 else 1)
                    if jt == NJT - 1:
                        # evict this block's PV accumulators (O and den rows)
                        for par in range(2):
                            h = 2 * pair + par
                            a, b = pv_t[2 * par], pv_t[2 * par + 1]
                            osl = ot_sb[h][:, ib * IB:(ib + 1) * IB]
                            nc.vector.tensor_copy(osl, a[0:DH + 1, :])
                            with nc.allow_low_precision(
                                    reason="bf16 j-half merge, 0.4% ok"):
                                nc.vector.tensor_add(osl, osl, b[0:DH + 1, :])
                        if pair == 1:
                            emit_chain(ib)
                emit_proj_half(NIB - 1, 0)
                emit_proj_half(NIB - 1, 1)

    nc.finalize()
    return nc


_nc_cache = {}


def _get_program(n_reps):
    if n_reps not in _nc_cache:
        _nc_cache[n_reps] = build_program(n_reps)
    return _nc_cache[n_reps]


def make_in_maps(x, w_qkv, w_out, b_out):
    x = np.asarray(x, np.float32)
    w_qkv = np.asarray(w_qkv, np.float32)
    w_out = np.asarray(w_out, np.float32)
    b_out = np.asarray(b_out, np.float32)
    in_maps = []
    for core in range(N_CORES):
        b, hg = core // 2, core % 2
        s = 256 * hg
        wq = w_qkv[s:s + 256]
        wk = w_qkv[512 + s:512 + s + 256]
        wv_ = w_qkv[1024 + s:1024 + s + 256]
        in_maps.append({
            "xT": np.ascontiguousarray(x[b].T).astype(BF16NP),
            "wqk": np.ascontiguousarray(np.concatenate([wq, wk], 0).T).astype(BF16NP),
            "wv": np.ascontiguousarray(wv_.T).astype(BF16NP),
            "wo": np.ascontiguousarray(w_out[:, s:s + 256].T).astype(BF16NP),
            "bias": np.ascontiguousarray((b_out / 2).reshape(D, 1)),
        })
    return in_maps


def kernel(x, w_qkv, w_out, b_out):
    nc = _get_program(N_REPS)
    in_maps = make_in_maps(x, w_qkv, w_out, b_out)
    res = run_bass_kernel_spmd(nc, in_maps, list(range(N_CORES)))
    out = np.empty((B, N, D), np.float32)
    for b in range(B):
        out[b] = (res.results[2 * b]["yT"] + res.results[2 * b + 1]["yT"]).T
    return out


if __name__ == "__main__":
    nc = build_program(1)
    print("built OK; instructions:",
          sum(len(blk.instructions) for f in nc.m.functions for blk in f.blocks))


# revision 34
# speedup vs baseline: 1.0294x; 1.0174x over previous
"""Multi-head attention (B=4, N=2048, D=512, H=8, Dh=64) on 8 trn2 cores.

Sharding: core c handles batch b = c//2 and head-group hg = c%2 (4 heads =
2 pairs).  Each core computes its batch's attention output for its 4 heads
plus the partial output projection; the host sums the two head-group
partials per batch.

v3: every matmul in the attention steady state runs in 64-row PE-tiling
mode (tiles T0/T8) so the array never pays a mode-switch drain (~105ns),
and row-tile pairs stream concurrently (2x) where the contraction is 64:
 - Scores S^T per pair: T0 computes the even head, T8 the odd head,
   concurrently, into the two banks of one [128, 1024] PSUM tile.
 - exp on the scalar engine per jt ([128, 1024], ~1020ns each).
 - PV splits its 128-contraction into T0/T8 j-halves (concurrent), with a
   [V|ones] 65-column stationary; the two half-results and half-denominators
   are combined by single fused DVE adds on PSUM eviction.
 - Denominators are batch-reciprocaled ([16,128] DVE op per i-block) and
   broadcast back via a DRAM bounce; normalization and the output
   projection for block ib run interleaved under block ib+1's attention.
All PE operands bf16 (f32 PSUM accumulation); QKV projection runs in full
128-row mode as a separate phase.
"""

import sys

for p in ("/opt/trn_rl_repo", "/root/.axon_site/_ro/trn_rl_repo"):
    if p not in sys.path:
        sys.path.insert(0, p)

from contextlib import ExitStack

import numpy as np
import ml_dtypes

import concourse.bass as bass
import concourse.mybir as mybir
import concourse.tile as tile
from concourse import bacc
from concourse.bass_utils import run_bass_kernel_spmd

F32 = mybir.dt.float32
BF16 = mybir.dt.bfloat16
AF = mybir.ActivationFunctionType
BF16NP = ml_dtypes.bfloat16

N_CORES = 8
B, N, D = 4, 2048, 512
HEADS = 8
DH = 64
SCALE = DH**-0.5
HPC = 4  # heads per core (2 pairs)
P = 128
NDT = D // P  # 4 d-tiles
NJT = N // P  # 16 j-tiles
IB = 512  # i-block
NIB = N // IB  # 4 i-blocks

N_REPS = 1  # replications of the whole body inside one NEFF (for timing)


def build_program(n_reps: int = N_REPS):
    nc = bacc.Bacc("TRN2", target_bir_lowering=False, debug=False,
                   num_devices=N_CORES)
    xT = nc.dram_tensor("xT", [D, N], BF16, kind="ExternalInput").ap()
    wqk = nc.dram_tensor("wqk", [D, 2 * HPC * DH], BF16, kind="ExternalInput").ap()
    wv = nc.dram_tensor("wv", [D, HPC * DH], BF16, kind="ExternalInput").ap()
    wo = nc.dram_tensor("wo", [HPC * DH, D], BF16, kind="ExternalInput").ap()
    bias = nc.dram_tensor("bias", [D, 1], F32, kind="ExternalInput").ap()
    yT = nc.dram_tensor("yT", [D, N], F32, kind="ExternalOutput").ap()
    # DRAM bounce for denominator reshape + partition broadcast
    den_d = nc.dram_tensor("den_d", [NIB, HPC * IB], BF16).ap()
    rden_d = nc.dram_tensor("rden_d", [NIB, HPC * IB], BF16).ap()


    with tile.TileContext(nc) as tc, ExitStack() as ctx:
        sb = ctx.enter_context(tc.tile_pool(name="sb", bufs=1))
        if n_reps > 1:
            ctx.enter_context(tc.For_i(0, n_reps, 1))

        for _rep in range(1):
            # preload the exp activation table while the input DMAs run
            warm = sb.tile([1, 16], F32, tag="warm", bufs=1)
            nc.vector.memset(warm, 0.0)
            nc.scalar.activation(warm, warm, AF.Exp, scale=1.0)

            # ---------------- phase 1: load + QKV projection ----------------
            wqk_sb = []
            wv_sb = []
            bias_sb = []
            for dt in range(NDT):
                t = sb.tile([P, 2 * HPC * DH], BF16, tag="wqk", bufs=NDT)
                nc.sync.dma_start(out=t, in_=wqk[dt * P:(dt + 1) * P, :])
                wqk_sb.append(t)
                t = sb.tile([P, HPC * DH], BF16, tag="wv", bufs=NDT)
                nc.sync.dma_start(out=t, in_=wv[dt * P:(dt + 1) * P, :])
                wv_sb.append(t)
                t = sb.tile([P, 1], F32, tag="bias", bufs=NDT)
                nc.sync.dma_start(out=t, in_=bias[dt * P:(dt + 1) * P, :])
                bias_sb.append(t)
            wo_sb = []
            for h in range(HPC):
                t = sb.tile([DH, D], BF16, tag="wo", bufs=HPC)
                nc.sync.dma_start(out=t, in_=wo[h * DH:(h + 1) * DH, :])
                wo_sb.append(t)

            xt_sb = []
            for dt in range(NDT):
                t = sb.tile([P, N], BF16, tag="xt", bufs=NDT)
                nc.sync.dma_start(out=t, in_=xT[dt * P:(dt + 1) * P, :])
                xt_sb.append(t)

            # Q^T/K^T tiles [128, N]; rows 0:64 even head of pair, 64:128 odd.
            # et: 0 = Q pair0, 1 = Q pair1, 2 = K pair0, 3 = K pair1
            # Serial prefix computes only what block (0,0) needs first (Q/K of
            # pair0 + V for the first 8 j-tiles); the rest is emitted inside
            # the attention stream via borrowed PSUM slots.
            qkt_sb = [sb.tile([P, N], BF16, tag="qkt", bufs=4, name=f"qkt{_e}")
                      for _e in range(4)]
            v_sb = [sb.tile([P, HPC * (DH + 1)], BF16, tag="v", bufs=NJT,
                            name=f"v{_n}") for _n in range(NJT)]

            def emit_v_copy(t, pv):
                nc.vector.tensor_copy(
                    t.rearrange("p (h c) -> p h c", c=DH + 1)[:, :, 0:DH],
                    pv.rearrange("p (h c) -> p h c", c=DH),
                )
                nc.vector.memset(
                    t.rearrange("p (h c) -> p h c", c=DH + 1)[:, :, DH:DH + 1],
                    1.0,
                )

            with tc.tile_pool(name="ps1", bufs=1, space="PSUM") as ps1:
                for et in (0, 2):
                    t = qkt_sb[et]
                    for nb in range(NIB):
                        pq = ps1.tile([P, IB], F32, tag="qk", bufs=4)
                        for dt in range(NDT):
                            nc.tensor.matmul(
                                pq,
                                lhsT=wqk_sb[dt][:, et * P:(et + 1) * P],
                                rhs=xt_sb[dt][:, nb * IB:(nb + 1) * IB],
                                start=(dt == 0), stop=(dt == NDT - 1),
                            )
                        nc.vector.tensor_copy(t[:, nb * IB:(nb + 1) * IB], pq)

                # V natural [n, e] with a ones column per head: [128, 4*65]
                for nt in range(NJT // 2):
                    t = v_sb[nt]
                    pv = ps1.tile([P, HPC * DH], F32, tag="v", bufs=2)
                    for dt in range(NDT):
                        nc.tensor.matmul(
                            pv,
                            lhsT=xt_sb[dt][:, nt * P:(nt + 1) * P],
                            rhs=wv_sb[dt],
                            start=(dt == 0), stop=(dt == NDT - 1),
                        )
                    emit_v_copy(t, pv)

            # ---------------- phase 2: attention (all 64-row mode) ----------
            # O^T per head [65, N]: rows 0:64 = O, row 64 = softmax
            # denominator (rides along in the PSUM eviction). Separate tiles
            # per head keep the dependency ranges narrow.
            ot_sb = [sb.tile([DH + 1, N], BF16, tag="ot", bufs=HPC,
                             name=f"ot{_h}") for _h in range(HPC)]

            def ot_o(h, ib):
                return ot_sb[h][0:DH, ib * IB:(ib + 1) * IB]

            units = [(ib, pair, jt) for ib in range(NIB)
                     for pair in range(2) for jt in range(NJT)]
            su_t = {}

            with tc.tile_pool(name="ps2", bufs=1, space="PSUM") as ps2:
                def emit_scores(k):
                    ib, pair, jt = units[k]
                    su = ps2.tile([P, 2 * IB], F32, tag="s", bufs=2,
                                  name=f"su{k}")
                    for par in range(2):
                        lo, hi = par * DH, (par + 1) * DH
                        nc.tensor.matmul(
                            su[:, par * IB:(par + 1) * IB],
                            lhsT=qkt_sb[2 + pair][lo:hi, jt * P:(jt + 1) * P],
                            rhs=qkt_sb[pair][lo:hi, ib * IB:(ib + 1) * IB],
                            start=True, stop=True,
                        )
                    su_t[k] = su

                def emit_proj_half(ibp, half):
                    # output projection for 2 of the 4 d-tiles of block ibp,
                    # borrowing a su-tag PSUM slot ([128, 1024] = 2 banks).
                    yp2 = ps2.tile([P, 2 * IB], F32, tag="s", bufs=2,
                                   name=f"yp{ibp}_{half}")
                    yt_t = sb.tile([P, 2 * IB], F32, tag="yt", bufs=2)
                    for i in range(2):
                        dt4 = 2 * half + i
                        for h in range(HPC):
                            nc.tensor.matmul(
                                yp2[:, i * IB:(i + 1) * IB],
                                lhsT=wo_sb[h][:, dt4 * P:(dt4 + 1) * P],
                                rhs=ot_o(h, ibp),
                                start=(h == 0), stop=(h == HPC - 1),
                            )
                        nc.vector.tensor_scalar_add(
                            yt_t[:, i * IB:(i + 1) * IB],
                            yp2[:, i * IB:(i + 1) * IB], bias_sb[dt4])
                        nc.sync.dma_start(
                            out=yT[dt4 * P:(dt4 + 1) * P,
                                   ibp * IB:(ibp + 1) * IB],
                            in_=yt_t[:, i * IB:(i + 1) * IB],
                        )

                def emit_chain(ib):
                    # denominator reciprocal + normalization for block ib:
                    # per-head den rows -> DRAM, reciprocal on [16,128], one
                    # broadcast DMA for all heads, gpsimd muls.
                    for h in range(HPC):
                        nc.sync.dma_start(
                            out=den_d[ib:ib + 1, h * IB:(h + 1) * IB],
                            in_=ot_sb[h][DH:DH + 1, ib * IB:(ib + 1) * IB])
                    rc2 = sb.tile([4 * HPC, P], BF16, tag="rc2", bufs=2)
                    nc.sync.dma_start(
                        out=rc2,
                        in_=den_d[ib:ib + 1, :].rearrange(
                            "a (q p) -> (a q) p", p=P))
                    with nc.allow_low_precision(
                            reason="bf16 reciprocal of softmax denom, 0.4% ok"):
                        nc.vector.reciprocal(rc2, rc2)
                    nc.sync.dma_start(
                        out=rden_d[ib:ib + 1, :].rearrange(
                            "a (q p) -> (a q) p", p=P),
                        in_=rc2)
                    for h in range(HPC):
                        rb = sb.tile([DH, IB], BF16, tag="rb", bufs=4)
                        src = rden_d[ib, h * IB:(h + 1) * IB]
                        bcast = bass.AP(
                            tensor=src.tensor, offset=src.offset,
                            ap=[[0, DH]] + [list(d) for d in src.ap[-1:]],
                        )
                        nc.sync.dma_start(out=rb, in_=bcast)
                        nc.gpsimd.tensor_mul(ot_o(h, ib), ot_o(h, ib), rb)

                def emit_v_insert(nts):
                    # V projection for 4 j-tiles in one borrowed su slot
                    slot = ps2.tile([P, 2 * IB], F32, tag="s", bufs=2,
                                    name=f"vi{nts[0]}")
                    W = HPC * DH
                    for i, nt in enumerate(nts):
                        for dt in range(NDT):
                            nc.tensor.matmul(
                                slot[:, i * W:(i + 1) * W],
                                lhsT=xt_sb[dt][:, nt * P:(nt + 1) * P],
                                rhs=wv_sb[dt],
                                start=(dt == 0), stop=(dt == NDT - 1),
                            )
                        emit_v_copy(v_sb[nt], slot[:, i * W:(i + 1) * W])

                def emit_qk_insert(items):
                    # Q/K projection for 2 (et, nb) pieces in one su slot
                    slot = ps2.tile([P, 2 * IB], F32, tag="s", bufs=2,
                                    name=f"qi{items[0][0]}_{items[0][1]}")
                    for i, (et, nb) in enumerate(items):
                        for dt in range(NDT):
                            nc.tensor.matmul(
                                slot[:, i * IB:(i + 1) * IB],
                                lhsT=wqk_sb[dt][:, et * P:(et + 1) * P],
                                rhs=xt_sb[dt][:, nb * IB:(nb + 1) * IB],
                                start=(dt == 0), stop=(dt == NDT - 1),
                            )
                        nc.vector.tensor_copy(
                            qkt_sb[et][:, nb * IB:(nb + 1) * IB],
                            slot[:, i * IB:(i + 1) * IB])

                inserts = {
                    2: lambda: emit_v_insert((8, 9, 10, 11)),
                    4: lambda: emit_v_insert((12, 13, 14, 15)),
                    6: lambda: emit_qk_insert(((1, 0), (1, 1))),
                    8: lambda: emit_qk_insert(((1, 2), (1, 3))),
                    10: lambda: emit_qk_insert(((3, 0), (3, 1))),
                    12: lambda: emit_qk_insert(((3, 2), (3, 3))),
                }

                emit_scores(0)
                pv_t = None
                for k, (ib, pair, jt) in enumerate(units):
                    if jt == 0:
                        pv_t = [ps2.tile([P, IB], F32, tag="pv", bufs=4,
                                         name=f"pv{ib}_{pair}_{_k}")
                                for _k in range(4)]
                    if k + 1 < len(units):
                        emit_scores(k + 1)
                    if k in inserts:
                        inserts[k]()
                    es = sb.tile([P, 2 * IB], BF16, tag="big", bufs=8)
                    nc.scalar.activation(es, su_t.pop(k), AF.Exp, scale=SCALE)
                    for par in range(2):
                        h = 2 * pair + par
                        for jh in range(2):
                            nc.tensor.matmul(
                                pv_t[2 * par + jh][0:DH + 1, :],
                                lhsT=v_sb[jt][jh * DH:(jh + 1) * DH,
                                              h * (DH + 1):(h + 1) * (DH + 1)],
                                rhs=es[jh * DH:(jh + 1) * DH,
                                       par * IB:(par + 1) * IB],
                                start=(jt == 0), stop=(jt == NJT - 1),
                            )
                    if pair == 0 and ib >= 1 and jt in (6, 11):
                        emit_proj_half(ib - 1, 0 if jt == 6 else 1)
                    if jt == NJT - 1:
                        # evict this block's PV accumulators (O and den rows)
                        for par in range(2):
                            h = 2 * pair + par
                            a, b = pv_t[2 * par], pv_t[2 * par + 1]
                            osl = ot_sb[h][:, ib * IB:(ib + 1) * IB]
                            nc.vector.tensor_copy(osl, a[0:DH + 1, :])
                            with nc.allow_low_precision(
                                    reason="bf16 j-half merge, 0.4% ok"):
                                nc.vector.tensor_add(osl, osl, b[0:DH + 1, :])
                        if pair == 1:
                            emit_chain(ib)
                emit_proj_half(NIB - 1, 0)
                emit_proj_half(NIB - 1, 1)

    nc.finalize()
    return nc


_nc_cache = {}


def _get_program(n_reps):
    if n_reps not in _nc_cache:
        _nc_cache[n_reps] = build_program(n_reps)
    return _nc_cache[n_reps]


def make_in_maps(x, w_qkv, w_out, b_out):
    x = np.asarray(x, np.float32)
    w_qkv = np.asarray(w_qkv, np.float32)
    w_out = np.asarray(w_out, np.float32)
    b_out = np.asarray(b_out, np.float32)
    in_maps = []
    for core in range(N_CORES):
        b, hg = core // 2, core % 2
        s = 256 * hg
        wq = w_qkv[s:s + 256]
        wk = w_qkv[512 + s:512 + s + 256]
        wv_ = w_qkv[1024 + s:1024 + s + 256]
        in_maps.append({
            "xT": np.ascontiguousarray(x[b].T).astype(BF16NP),
            "wqk": np.ascontiguousarray(np.concatenate([wq, wk], 0).T).astype(BF16NP),
            "wv": np.ascontiguousarray(wv_.T).astype(BF16NP),
            "wo": np.ascontiguousarray(w_out[:, s:s + 256].T).astype(BF16NP),
            "bias": np.ascontiguousarray((b_out / 2).reshape(D, 1)),
        })
    return in_maps


def kernel(x, w_qkv, w_out, b_out):
    nc = _get_program(N_REPS)
    in_maps = make_in_maps(x, w_qkv, w_out, b_out)
    res = run_bass_kernel_spmd(nc, in_maps, list(range(N_CORES)))
    out = np.empty((B, N, D), np.float32)
    for b in range(B):
        out[b] = (res.results[2 * b]["yT"] + res.results[2 * b + 1]["yT"]).T
    return out


if __name__ == "__main__":
    nc = build_program(1)
    print("built OK; instructions:",
          sum(len(blk.instructions) for f in nc.m.functions for blk in f.blocks))
